# revision 1
# baseline (speedup 1.0000x reference)
"""GCN encoder kernel for Trainium2, SPMD across 8 NeuronCores.

Computes (reference semantics):
    x_ = P @ (x @ W1 + b1)
    h  = P @ (1.8 * l2norm_rows(x @ W2 + b2))
where P = D^-1/2 (A + I) D^-1/2 over the edge list (by destination).

Strategy:
  * Both branches concatenated into one feature matrix u[N, 256] (bf16),
    pre-scaled by dinv[src]; computed replicated on every core (phase A).
  * Edges partitioned by destination across 8 cores, sorted by dst,
    grouped into 128-dst windows.  Per window: indirect-DMA gather of
    source rows, one-hot slot matrix S built on DVE, segment-sum via
    PE matmul accumulation in PSUM, final dinv[dst] scale (phase B).
  * Self loops are injected as regular edges on the host; window edge
    lists are padded to a uniform T tiles of 128 with slot=-1 sentinels
    (an S column of zeros kills their contribution).
"""
import sys

import numpy as np

try:
    import concourse.bass as bass  # noqa: F401
except ImportError:
    sys.path.insert(0, "/opt/trn_rl_repo")

from contextlib import ExitStack

from ml_dtypes import bfloat16

import concourse.bass as bass
import concourse.bacc as bacc
import concourse.tile as tile
from concourse import mybir
from concourse.bass_utils import run_bass_kernel_spmd

N_CORES = 8
WIN = 128
ROWG = 512  # phase-A row group (per xT load)
NSEG = 4   # u-row segments for int16 dma_gather indices (seg size < 32768)
MSG_BUFS = 4  # message-tile double buffering (in-flight gather windows)
FORCE_TSEG = None  # debug: pad every segment's tile count up to this


def _dims(n_nodes, c_in, c_out):
    npc = n_nodes // N_CORES
    nwin = -(-npc // WIN)
    u_rows = -(-n_nodes // ROWG) * ROWG
    seg = u_rows // NSEG
    assert seg % 128 == 0 and seg <= 32768
    return dict(
        N=n_nodes, CIN=c_in, COUT=c_out, C=2 * c_out, NPC=npc, NWIN=nwin,
        NGW=N_CORES * nwin, U_ROWS=u_rows, G=u_rows // ROWG,
        KCH=c_in // 128, SEG=seg,
    )


def _prep(x, edge_index, W1, b1, W2, b2):
    """Host-side sharding: degrees, edge partitioning/packing, dtype prep."""
    x = np.asarray(x, np.float32)
    n, c_in = x.shape
    c_out = W1.shape[1]
    d = _dims(n, c_in, c_out)
    e = edge_index.shape[1]

    src = np.asarray(edge_index[0], np.int64)
    dst = np.asarray(edge_index[1], np.int64)

    deg = (np.bincount(dst, minlength=n) + 1).astype(np.float32)
    dinv = (1.0 / np.sqrt(deg)).astype(np.float32)

    # combined edge list: real edges + self loops
    n_all = np.arange(n, dtype=np.int64)
    src_a = np.concatenate([src, n_all])
    dst_a = np.concatenate([dst, n_all])
    gw_a = (dst_a // d["NPC"]) * d["NWIN"] + (dst_a % d["NPC"]) // WIN
    seg_a = src_a // d["SEG"]
    slot_a = (dst_a % d["NPC"]) % WIN
    # order edges by (window, src-segment, src) — ascending src within a
    # gather call gives the DMA engines better HBM locality
    order = np.lexsort((src_a, seg_a, gw_a))
    src_o = src_a[order]
    gw_o = gw_a[order]
    seg_o = seg_a[order]
    slot_o = slot_a[order].astype(np.float32)
    grp_o = gw_o * NSEG + seg_o  # non-decreasing
    ngrp = d["NGW"] * NSEG
    cnt = np.bincount(grp_o, minlength=ngrp).astype(np.int64)
    # per-segment tile count: max over all (core, window)
    t_seg = [int(-(-cnt.reshape(-1, NSEG)[:, s].max() // WIN))
             for s in range(NSEG)]
    t_seg = [max(t, 1) for t in t_seg]
    if FORCE_TSEG is not None:
        t_seg = [max(t, FORCE_TSEG) for t in t_seg]
    off = np.concatenate([[0], np.cumsum(t_seg)]).astype(np.int64)
    tt = int(off[-1])  # total tiles per window
    d["T_SEG"] = t_seg
    d["TT"] = tt

    # positions within each (window, segment) group
    start = np.zeros(ngrp, np.int64)
    start[1:] = np.cumsum(cnt)[:-1]
    pos = np.arange(src_o.shape[0], dtype=np.int64) - start[grp_o]
    # logical index within the window's tile stream
    li = off[seg_o] * WIN + pos
    # slot array: dest layout (128-wrap): [gw, li%128, li//128]
    slot_buf = np.full((d["NGW"], WIN, tt), -1.0, np.float32)
    slot_buf[gw_o, li % WIN, li // WIN] = slot_o
    # index array: snake-16 layout [gw, li%16, li//16], then replicate x8.
    # Padding entries gather row 0 of the segment (slot=-1 kills their
    # contribution in S), keeping the valid-index count static across cores.
    idx16 = np.zeros((d["NGW"], 16, tt * 8), np.int16)
    idx16[gw_o, li % 16, li // 16] = (src_o - seg_o * d["SEG"]).astype(np.int16)
    idx_buf = np.tile(idx16, (1, 8, 1))  # [NGW, 128, tt*8]

    dinv_pad = np.ones(d["U_ROWS"], np.float32)
    dinv_pad[:n] = dinv
    dinvr = np.ascontiguousarray(dinv_pad.reshape(-1, 128).T)  # [128, U_ROWS/128]

    dd = np.ones((N_CORES, d["NWIN"] * WIN), np.float32)
    dd[:, :d["NPC"]] = dinv.reshape(N_CORES, d["NPC"])
    dinvd = np.ascontiguousarray(
        dd.reshape(N_CORES, d["NWIN"], WIN).transpose(0, 2, 1))  # [8,128,NWIN]

    xt = np.zeros((c_in, d["U_ROWS"]), bfloat16)
    xt[:, :n] = x.T.astype(bfloat16)
    wc = np.concatenate([W1, W2], axis=1).astype(bfloat16)  # [CIN, 2*COUT]
    bias_bc = np.tile(
        np.concatenate([b1, b2]).astype(np.float32)[None, :], (128, 1))
    iota_bc = np.tile(np.arange(128, dtype=np.float32)[None, :], (128, 1))

    in_maps = []
    for k in range(N_CORES):
        in_maps.append({
            "xt": xt,
            "wc": wc,
            "bias": bias_bc,
            "iota": iota_bc,
            "dinvr": dinvr,
            "dinvd": dinvd[k],
            "edi": idx_buf[k * d["NWIN"]:(k + 1) * d["NWIN"]],
            "eds": slot_buf[k * d["NWIN"]:(k + 1) * d["NWIN"]],
        })
    return in_maps, d


def _build(d):
    """Emit the SPMD Bass program (identical on all cores; data differs)."""
    f32, bf16 = mybir.dt.float32, mybir.dt.bfloat16
    i16 = mybir.dt.int16
    C, CIN, KCH = d["C"], d["CIN"], d["KCH"]
    TT, T_SEG, SEG = d["TT"], d["T_SEG"], d["SEG"]
    nrt = d["U_ROWS"] // 128  # number of phase-A row tiles

    nc = bacc.Bacc("TRN2", target_bir_lowering=False, debug=False,
                   num_swdge_queues=4)
    xt_d = nc.dram_tensor("xt", [CIN, d["U_ROWS"]], bf16, kind="ExternalInput")
    wc_d = nc.dram_tensor("wc", [CIN, C], bf16, kind="ExternalInput")
    bias_d = nc.dram_tensor("bias", [128, C], f32, kind="ExternalInput")
    iota_d = nc.dram_tensor("iota", [128, 128], f32, kind="ExternalInput")
    dinvr_d = nc.dram_tensor("dinvr", [128, nrt], f32, kind="ExternalInput")
    dinvd_d = nc.dram_tensor("dinvd", [128, d["NWIN"]], f32, kind="ExternalInput")
    edi_d = nc.dram_tensor("edi", [d["NWIN"], 128, TT * 8], i16,
                           kind="ExternalInput")
    eds_d = nc.dram_tensor("eds", [d["NWIN"], 128, TT], f32,
                           kind="ExternalInput")
    out_d = nc.dram_tensor("out", [d["NPC"], C], f32, kind="ExternalOutput")
    u_d = nc.dram_tensor("u", [d["U_ROWS"], C], bf16)  # internal scratch

    with ExitStack() as ctx:
        tc = ctx.enter_context(tile.TileContext(nc))
        const_p = ctx.enter_context(tc.tile_pool(name="const", bufs=1))
        xa_p = ctx.enter_context(tc.tile_pool(name="xa", bufs=4))
        ta_p = ctx.enter_context(tc.tile_pool(name="ta", bufs=6))
        sq_p = ctx.enter_context(tc.tile_pool(name="sq", bufs=4))
        col_p = ctx.enter_context(tc.tile_pool(name="col", bufs=16))
        ua_p = ctx.enter_context(tc.tile_pool(name="ua", bufs=6))
        ed_p = ctx.enter_context(tc.tile_pool(name="ed", bufs=4))
        msg_p = ctx.enter_context(tc.tile_pool(name="msg", bufs=MSG_BUFS))
        s_p = ctx.enter_context(tc.tile_pool(name="s", bufs=56))
        out_p = ctx.enter_context(tc.tile_pool(name="o", bufs=3))
        psa_p = ctx.enter_context(tc.tile_pool(name="psa", bufs=4, space="PSUM"))
        psb_p = ctx.enter_context(tc.tile_pool(name="psb", bufs=2, space="PSUM"))

        # constants
        wc_t = [const_p.tile([128, C], bf16, name=f"wct{kc}", tag=f"wc{kc}")
                for kc in range(KCH)]
        for kc in range(KCH):
            nc.sync.dma_start(out=wc_t[kc][:], in_=wc_d[kc * 128:(kc + 1) * 128, :])
        bias_t = const_p.tile([128, C], f32)
        nc.sync.dma_start(out=bias_t[:], in_=bias_d[:, :])
        iota_t = const_p.tile([128, 128], f32)
        nc.sync.dma_start(out=iota_t[:], in_=iota_d[:, :])
        dinvr_t = const_p.tile([128, nrt], f32)
        nc.sync.dma_start(out=dinvr_t[:], in_=dinvr_d[:, :])
        dinvd_t = const_p.tile([128, d["NWIN"]], f32)
        nc.sync.dma_start(out=dinvd_t[:], in_=dinvd_d[:, :])
        eps_t = const_p.tile([128, 1], f32)
        nc.vector.memset(eps_t[:], 1e-24)
        zeros_t = const_p.tile([128, 128], f32)
        nc.vector.memset(zeros_t[:], 0.0)

        co = d["COUT"]
        inv_s2 = 1.0 / (1.8 * 1.8)

        # ---- phase A: u[r] = [dinv*(x@W1+b1) | dinv*1.8*l2n(x@W2+b2)] ----
        for g in range(d["G"]):
            xg = [xa_p.tile([128, ROWG], bf16, name=f"xg{kc}", tag=f"xg{kc}")
                  for kc in range(KCH)]
            for kc in range(KCH):
                nc.sync.dma_start(
                    out=xg[kc][:],
                    in_=xt_d[kc * 128:(kc + 1) * 128, g * ROWG:(g + 1) * ROWG])
            for jj in range(ROWG // 128):
                rt = g * (ROWG // 128) + jj
                ps = psa_p.tile([128, C], f32)
                for kc in range(KCH):
                    nc.tensor.matmul(
                        ps[:], lhsT=xg[kc][:, jj * 128:(jj + 1) * 128],
                        rhs=wc_t[kc][:], start=(kc == 0), stop=(kc == KCH - 1))
                t_t = ta_p.tile([128, C], f32)
                nc.vector.tensor_tensor(
                    out=t_t[:], in0=ps[:], in1=bias_t[:], op=mybir.AluOpType.add)
                sq_t = sq_p.tile([128, co], f32)
                s_col = col_p.tile([128, 1], f32, tag="scol")
                nc.vector.scalar_tensor_tensor(
                    out=sq_t[:], in0=t_t[:, co:], scalar=1.0, in1=t_t[:, co:],
                    op0=mybir.AluOpType.mult, op1=mybir.AluOpType.mult,
                    accum_out=s_col[:])
                nrm = col_p.tile([128, 1], f32, tag="nrm")
                nc.scalar.activation(
                    out=nrm[:], in_=s_col[:],
                    func=mybir.ActivationFunctionType.Sqrt,
                    bias=eps_t[:], scale=inv_s2)
                rn = col_p.tile([128, 1], f32, tag="rn")
                nc.vector.reciprocal(out=rn[:], in_=nrm[:])
                phi2 = col_p.tile([128, 1], f32, tag="phi2")
                nc.vector.tensor_tensor(
                    out=phi2[:], in0=rn[:], in1=dinvr_t[:, rt:rt + 1],
                    op=mybir.AluOpType.mult)
                u_t = ua_p.tile([128, C], bf16)
                nc.scalar.activation(
                    out=u_t[:, :co], in_=t_t[:, :co],
                    func=mybir.ActivationFunctionType.Copy,
                    bias=0.0, scale=dinvr_t[:, rt:rt + 1])
                nc.scalar.activation(
                    out=u_t[:, co:], in_=t_t[:, co:],
                    func=mybir.ActivationFunctionType.Copy,
                    bias=0.0, scale=phi2[:])
                nc.sync.dma_start(
                    out=u_d[rt * 128:(rt + 1) * 128, :], in_=u_t[:])

        # ---- phase B: per dst window, segmented gather + segment matmul ----
        seg_off = [0]
        for t in T_SEG:
            seg_off.append(seg_off[-1] + t)
        for w in range(d["NWIN"]):
            ei_t = ed_p.tile([128, TT * 8], i16, tag="ei")
            nc.sync.dma_start(out=ei_t[:], in_=edi_d[w, :, :])
            es_t = ed_p.tile([128, TT], f32, tag="es")
            nc.sync.dma_start(out=es_t[:], in_=eds_d[w, :, :])
            msg_t = msg_p.tile([128, TT, C], bf16)
            for s in range(NSEG):
                ts = T_SEG[s]
                o0 = seg_off[s]
                lo = s * SEG
                hi = min(d["U_ROWS"], (s + 1) * SEG)
                # single_packet=False lifts the 64-desc/packet (1024 idx) cap
                nc.gpsimd.dma_gather(
                    out_ap=msg_t[:, o0:o0 + ts, :],
                    in_ap=u_d[lo:hi, :],
                    idxs_ap=ei_t[:, o0 * 8:(o0 + ts) * 8],
                    num_idxs=ts * 128,
                    num_idxs_reg=ts * 128,
                    elem_size=C,
                    single_packet=False,
                    queue_num=s % 4)
            # two interleaved PSUM accumulation chains to break the serial
            # per-window matmul dependency chain
            ps0 = psb_p.tile([128, C], f32, name="ps0", tag="ps0")
            ps1 = psb_p.tile([128, C], f32, name="ps1", tag="ps1")
            n0 = (TT + 1) // 2  # tiles going to ps0 (even t)
            n1 = TT - n0
            c0 = c1 = 0
            for t in range(TT):
                s_t = s_p.tile([128, 128], bf16)
                nc.vector.scalar_tensor_tensor(
                    out=s_t[:], in0=iota_t[:], scalar=es_t[:, t:t + 1],
                    in1=zeros_t[:], op0=mybir.AluOpType.subtract,
                    op1=mybir.AluOpType.is_equal)
                if t % 2 == 0:
                    nc.tensor.matmul(
                        ps0[:], lhsT=s_t[:], rhs=msg_t[:, t, :],
                        start=(c0 == 0), stop=(c0 == n0 - 1))
                    c0 += 1
                else:
                    nc.tensor.matmul(
                        ps1[:], lhsT=s_t[:], rhs=msg_t[:, t, :],
                        start=(c1 == 0), stop=(c1 == n1 - 1))
                    c1 += 1
            p1s = out_p.tile([128, C], f32, name="p1s", tag="p1s")
            nc.scalar.activation(
                out=p1s[:], in_=ps1[:],
                func=mybir.ActivationFunctionType.Copy,
                bias=0.0, scale=dinvd_t[:, w:w + 1])
            o_t = out_p.tile([128, C], f32)
            nc.vector.scalar_tensor_tensor(
                out=o_t[:], in0=ps0[:], scalar=dinvd_t[:, w:w + 1],
                in1=p1s[:], op0=mybir.AluOpType.mult,
                op1=mybir.AluOpType.add)
            rows = min(d["NPC"] - w * 128, 128)
            nc.sync.dma_start(
                out=out_d[w * 128:w * 128 + rows, :], in_=o_t[:rows, :])

    nc.compile()
    return nc


def _run(in_maps, d, trace=False):
    nc = _build(d)
    res = run_bass_kernel_spmd(
        nc, in_maps, core_ids=list(range(N_CORES)), trace=trace)
    outs = np.concatenate(
        [res.results[k]["out"] for k in range(N_CORES)], axis=0)
    co = d["COUT"]
    x_ = np.ascontiguousarray(outs[:, :co])
    h = np.ascontiguousarray(outs[:, co:])
    return (h, x_), res


def kernel(x, edge_index, W1, b1, W2, b2):
    in_maps, d = _prep(x, edge_index, W1, b1, W2, b2)
    (h, x_), _ = _run(in_maps, d, trace=False)
    return (h, x_)

